# revision 8
# baseline (speedup 1.0000x reference)
"""Trainium2 Bass kernel for nn_ModalityConsisLoss (8 NeuronCores, data-parallel).

Reference computation:
    v_spa/v_seq = concat([f[:,a,:], f[:,2,:]], -1) @ W + b   for a in (0,1,3)  -> [3B, D]
    z = normalize_rows(concat([v_spa, v_seq]))               -> [6B, D]
    sim = z @ z.T ;  pos = diag pairs (i, i+3B)
    loss = sum(-pos/T) + sum(log(rowsum(exp(sim/T)) - diag)) / (6B)

Strategy (data-parallel over B):
  Each core owns B/8 = 256 batch rows -> 1536 of the 12288 z-rows
  (rows of both modalities for its batch slice, so pos pairs stay local).
  Per core, on device:
    - load f shards + W, cast bf16, PE-transpose f -> fT (d_in on partitions)
    - projection matmuls -> vT [512(d), 1536(rows)] fp32 (d_out on partitions)
    - column norms via ones-matmul; r = exp(-0.5*ln(ssq)) (one ACT table set)
    - zT_local = bf16(vT * r)  [512, 1536]
    - AllGather zT (bf16, 12.6 MB total) across the 8 cores
    - sim tiles: zT_local.T @ zT_all with fused exp(2*sim) + row-sum on ACT
    - denom = rowsum - e^2 ; partial loss = sum(log denom) - (2/T)*sum(pos)
  Host sums the 8 partial scalars (the trivial all-reduce of the loss).
"""
import sys
from contextlib import ExitStack

sys.path.insert(0, "/opt/trn_rl_repo")

import numpy as np

import concourse.bass as bass
import concourse.mybir as mybir
import concourse.tile as tile
from concourse import bacc
from concourse import bass_utils
from concourse.masks import make_identity

F32 = mybir.dt.float32
BF16 = mybir.dt.bfloat16
AF = mybir.ActivationFunctionType
ALU = mybir.AluOpType

N_CORES = 8
B = 2048
BL = B // N_CORES          # 256 local batch rows
D = 512
KB = D // 128              # 4 d blocks of 128
LROWS = 6 * BL             # 1536 local z-rows (spa 768 | seq 768)
RANK_COLS = LROWS          # columns contributed per rank to zT_all
R = N_CORES * LROWS        # 12288 total rows
IB = LROWS // 128          # 12 row blocks of 128 per core
LH = (0, 1, 3)             # left heads of the pairs (x, 2)
TEMP = 0.5
ESCALE = 1.0 / TEMP        # exp(sim/T) = exp(2*sim)
E2 = float(np.exp(2.0))    # diagonal term exp(2 * ||z||^2), ||z|| == 1
INV_COUNT = 1.0 / R        # final 1/(2*half)


def _body(ctx, nc, tc, f_aps, w_ap, b_ap, out_ap):
    const_pool = ctx.enter_context(tc.tile_pool(name="const", bufs=1))
    if True:
        ident = const_pool.tile([128, 128], F32)
        make_identity(nc, ident[:])
        ones_col = const_pool.tile([128, 1], F32)
        nc.vector.memset(ones_col[:], 1.0)
        ones_row = const_pool.tile([1, 128], F32)
        nc.vector.memset(ones_row[:], 1.0)
        neg_e2 = const_pool.tile([128, 1], F32)
        nc.vector.memset(neg_e2[:], -E2)

        # b columns: [128, 4] (per d_out block)
        b_col = const_pool.tile([128, 4], F32)
        for m in range(KB):
            nc.sync.dma_start(b_col[:, m:m + 1], b_ap[m * 128:(m + 1) * 128])

        w_bf = const_pool.tile([128, 8, D], BF16)

        vt_pool = ctx.enter_context(tc.tile_pool(name="vt", bufs=1))
        vT = vt_pool.tile([128, KB, LROWS], F32)   # [d_out(blk,128), rows]

        # ---------- projection: f -> vT ----------
        with tc.tile_pool(name="fstage", bufs=2) as fst_pool, \
             tc.tile_pool(name="ftrans", bufs=1) as ft_pool, \
             tc.tile_pool(name="ps_t", bufs=4, space="PSUM") as ps_t, \
             tc.tile_pool(name="ps_proj", bufs=2, space="PSUM") as ps_proj:
            # W: [1024, 512] f32 -> bf16 [128, 8(kblk), 512(d_out)]
            w_st = fst_pool.tile([128, 8, D], F32, tag="w_st")
            for kb in range(8):
                nc.sync.dma_start(w_st[:, kb, :],
                                  w_ap[kb * 128:(kb + 1) * 128, :])
            nc.vector.tensor_copy(w_bf[:], w_st[:])
            for mod in range(2):                   # 0 = spa, 1 = seq
                fT = ft_pool.tile([128, 4, KB, 2 * 128], BF16, name=f"fT{mod}",
                                  tag=f"fT{mod}")
                for h in range(2):                 # halves of 256 local rows
                    f_st = fst_pool.tile([128, 4 * D], F32, name="f_st",
                                         tag="f_st")
                    nc.sync.dma_start(
                        f_st[:], f_aps[mod][h * 128:(h + 1) * 128, :, :])
                    for a in range(4):
                        for kb in range(KB):
                            pst = ps_t.tile([128, 128], F32, name="pst",
                                            tag="pst")
                            nc.tensor.transpose(
                                pst[:],
                                f_st[:, a * D + kb * 128: a * D + (kb + 1) * 128],
                                ident[:])
                            nc.any.tensor_copy(
                                fT[:, a, kb, h * 128:(h + 1) * 128], pst[:])
                for pa in range(3):
                    for m in range(KB):
                        psv = ps_proj.tile([128, 2 * 128], F32, name="psv",
                                           tag="psv")
                        for kk in range(8):
                            head = LH[pa] if kk < 4 else 2
                            kb = kk % 4
                            nc.tensor.matmul(
                                psv[:],
                                lhsT=w_bf[:, kk, m * 128:(m + 1) * 128],
                                rhs=fT[:, head, kb, :],
                                start=(kk == 0), stop=(kk == 7))
                        col0 = mod * 768 + pa * 256
                        nc.vector.tensor_scalar_add(
                            vT[:, m, col0:col0 + 256], psv[:], b_col[:, m:m + 1])

        # ---------- norms, r, zT_local, pos ----------
        small_pool = ctx.enter_context(tc.tile_pool(name="small", bufs=1))
        zT_loc = small_pool.tile([128, KB, LROWS], BF16, tag="zT_loc")
        ssq = small_pool.tile([1, LROWS], F32, tag="ssq")
        with tc.tile_pool(name="sq", bufs=2) as sq_pool, \
             tc.tile_pool(name="ps_s", bufs=2, space="PSUM") as ps_s, \
             tc.tile_pool(name="ps_rb", bufs=2, space="PSUM") as ps_rb:
            for cc in range(3):
                ps_ssq = ps_s.tile([1, 512], F32, name="ps_ssq", tag="ps_s")
                for m in range(KB):
                    sq = sq_pool.tile([128, 512], F32, name="sq", tag="sq")
                    nc.vector.tensor_mul(sq[:], vT[:, m, cc * 512:(cc + 1) * 512],
                                         vT[:, m, cc * 512:(cc + 1) * 512])
                    nc.tensor.matmul(ps_ssq[:], lhsT=ones_col[:], rhs=sq[:],
                                     start=(m == 0), stop=(m == KB - 1))
                nc.vector.tensor_copy(ssq[:, cc * 512:(cc + 1) * 512], ps_ssq[:])

            lnss = small_pool.tile([1, LROWS], F32, tag="lnss")
            nc.scalar.activation(lnss[:], ssq[:], AF.Ln)
            r_row = small_pool.tile([1, LROWS], F32, tag="r_row")
            nc.scalar.activation(r_row[:], lnss[:], AF.Exp, scale=-0.5)

            for cc in range(3):
                rb = ps_rb.tile([128, 512], F32, name="rb", tag="rb")
                nc.tensor.matmul(rb[:], lhsT=ones_row[:],
                                 rhs=r_row[:, cc * 512:(cc + 1) * 512],
                                 start=True, stop=True)
                for m in range(KB):
                    nc.vector.tensor_mul(zT_loc[:, m, cc * 512:(cc + 1) * 512],
                                         vT[:, m, cc * 512:(cc + 1) * 512], rb[:])

            # pos_i = r_i * r_{i+768} * sum_d vT[d, i] * vT[d, i+768]
            pos_raw = small_pool.tile([1, 768], F32, tag="pos_raw")
            for c2 in range(2):
                ps_pp = ps_s.tile([1, 384], F32, name="ps_pp", tag="ps_s")
                for m in range(KB):
                    pp = sq_pool.tile([128, 384], F32, name="pp", tag="sq")
                    nc.vector.tensor_mul(
                        pp[:], vT[:, m, c2 * 384:(c2 + 1) * 384],
                        vT[:, m, 768 + c2 * 384:768 + (c2 + 1) * 384])
                    nc.tensor.matmul(ps_pp[:], lhsT=ones_col[:], rhs=pp[:],
                                     start=(m == 0), stop=(m == KB - 1))
                nc.vector.tensor_copy(pos_raw[:, c2 * 384:(c2 + 1) * 384],
                                      ps_pp[:])
            rrp = small_pool.tile([1, 768], F32, tag="rrp")
            nc.vector.tensor_mul(rrp[:], r_row[:, 0:768], r_row[:, 768:LROWS])
            pos_row = small_pool.tile([1, 768], F32, tag="pos_row")
            nc.vector.tensor_mul(pos_row[:], pos_raw[:], rrp[:])
            pos_sum = small_pool.tile([1, 1], F32, tag="pos_sum")
            nc.vector.tensor_reduce(pos_sum[:], pos_row[:],
                                    axis=mybir.AxisListType.X, op=ALU.add)

        # ---------- AllGather zT ----------
        dram_pool = ctx.enter_context(tc.tile_pool(name="dram", bufs=1,
                                                   space="DRAM"))
        big_pool = ctx.enter_context(tc.tile_pool(name="big", bufs=1))
        if True:
            ag_in = dram_pool.tile([4 * 128, RANK_COLS], BF16, tag="ag_in")
            ag_out = dram_pool.tile([N_CORES * 4 * 128, RANK_COLS], BF16,
                                    addr_space="Shared", tag="ag_out")
            for m in range(KB):
                nc.sync.dma_start(ag_in[m * 128:(m + 1) * 128, :],
                                  zT_loc[:, m, :])
            nc.gpsimd.collective_compute(
                "AllGather", ALU.bypass,
                replica_groups=[list(range(N_CORES))],
                ins=[ag_in.opt()], outs=[ag_out.opt()])

            zT_all = big_pool.tile([128, KB, R], BF16, tag="zT_all")
            for rr in range(N_CORES):
                for m in range(KB):
                    nc.sync.dma_start(
                        zT_all[:, m, rr * RANK_COLS:(rr + 1) * RANK_COLS],
                        ag_out[rr * 512 + m * 128: rr * 512 + (m + 1) * 128, :])

        # ---------- sim tiles + fused exp/rowsum ----------
        stats = small_pool.tile([128, IB * N_CORES], F32, tag="stats")
        with tc.tile_pool(name="ps_sim", bufs=2, space="PSUM") as ps_sim:
            for ib in range(IB):
                for rr in range(N_CORES):
                    ps = ps_sim.tile([128, RANK_COLS], F32, name="ps_sim",
                                     tag="ps_sim")
                    for jt in range(3):
                        for k in range(KB):
                            nc.tensor.matmul(
                                ps[:, jt * 512:(jt + 1) * 512],
                                lhsT=zT_loc[:, k, ib * 128:(ib + 1) * 128],
                                rhs=zT_all[:, k,
                                           rr * RANK_COLS + jt * 512:
                                           rr * RANK_COLS + (jt + 1) * 512],
                                start=(k == 0), stop=(k == KB - 1))
                    nc.scalar.activation(
                        ps[:], ps[:], AF.Exp, scale=ESCALE,
                        accum_out=stats[:, ib * N_CORES + rr:
                                        ib * N_CORES + rr + 1])

        # ---------- final reduction ----------
        with tc.tile_pool(name="ps_fin", bufs=1, space="PSUM") as ps_fin:
            denom = small_pool.tile([128, IB], F32, tag="denom")
            nc.vector.tensor_reduce(
                denom[:], stats.rearrange("p (i r) -> p i r", r=N_CORES),
                axis=mybir.AxisListType.X, op=ALU.add)
            logd = small_pool.tile([128, IB], F32, tag="logd")
            nc.scalar.activation(logd[:], denom[:], AF.Ln, bias=neg_e2[:])
            logsum = small_pool.tile([128, 1], F32, tag="logsum")
            nc.vector.tensor_reduce(logsum[:], logd[:],
                                    axis=mybir.AxisListType.X, op=ALU.add)
            fin = ps_fin.tile([1, 1], F32, tag="fin")
            nc.tensor.matmul(fin[:], lhsT=ones_col[:], rhs=logsum[:],
                             start=True, stop=True)
            res = small_pool.tile([1, 1], F32, tag="res")
            # res = (pos_sum * (-2/T) + sum(log denom)) / R
            nc.vector.scalar_tensor_tensor(res[:], pos_sum[:], -2.0 / TEMP,
                                           fin[:], op0=ALU.mult, op1=ALU.add)
            nc.vector.tensor_scalar_mul(res[:], res[:], INV_COUNT)
            nc.sync.dma_start(out_ap[:], res[:])


_NC_CACHE = None


def build_nc():
    global _NC_CACHE
    if _NC_CACHE is not None:
        return _NC_CACHE
    nc = bacc.Bacc("TRN2", target_bir_lowering=False, debug=False,
                   num_devices=N_CORES)
    f_spa = nc.dram_tensor("f_spa", [BL, 4, D], F32, kind="ExternalInput").ap()
    f_seq = nc.dram_tensor("f_seq", [BL, 4, D], F32, kind="ExternalInput").ap()
    w_ap = nc.dram_tensor("W", [2 * D, D], F32, kind="ExternalInput").ap()
    b_ap = nc.dram_tensor("b", [D], F32, kind="ExternalInput").ap()
    out_ap = nc.dram_tensor("out", [1, 1], F32, kind="ExternalOutput").ap()
    with tile.TileContext(nc) as tc, ExitStack() as ctx:
        _body(ctx, nc, tc, (f_spa, f_seq), w_ap, b_ap, out_ap)
    nc.compile()
    _NC_CACHE = nc
    return nc


def run(inputs, **kw):
    nc = build_nc()
    f_seq = np.ascontiguousarray(np.asarray(inputs["f_seq"], dtype=np.float32))
    f_spa = np.ascontiguousarray(np.asarray(inputs["f_spa"], dtype=np.float32))
    W = np.ascontiguousarray(np.asarray(inputs["W"], dtype=np.float32))
    b = np.ascontiguousarray(np.asarray(inputs["b"], dtype=np.float32))
    in_maps = []
    for c in range(N_CORES):
        sl = slice(c * BL, (c + 1) * BL)
        in_maps.append({"f_seq": np.ascontiguousarray(f_seq[sl]),
                        "f_spa": np.ascontiguousarray(f_spa[sl]),
                        "W": W, "b": b})
    res = bass_utils.run_bass_kernel_spmd(
        nc, in_maps, core_ids=list(range(N_CORES)), **kw)
    total = np.float64(0.0)
    for c in range(N_CORES):
        total += np.float64(res.results[c]["out"][0, 0])
    return np.float32(total), res


def kernel(**inputs) -> np.ndarray:
    loss, _ = run(inputs)
    return np.asarray(loss, dtype=np.float32)


if __name__ == "__main__":
    rng = np.random.default_rng(0)
    inputs = {
        "f_seq": rng.standard_normal((B, 4, D), dtype=np.float32),
        "f_spa": rng.standard_normal((B, 4, D), dtype=np.float32),
        "W": (rng.standard_normal((2 * D, D), dtype=np.float32) * 0.02),
        "b": np.zeros((D,), dtype=np.float32),
    }
    print(kernel(**inputs))


# revision 9
# speedup vs baseline: 1.6386x; 1.6386x over previous
"""Trainium2 Bass kernel for nn_ModalityConsisLoss (8 NeuronCores, data-parallel).

Reference computation:
    v_spa/v_seq = concat([f[:,a,:], f[:,2,:]], -1) @ W + b   for a in (0,1,3)  -> [3B, D]
    z = normalize_rows(concat([v_spa, v_seq]))               -> [6B, D]
    sim = z @ z.T ;  pos = diag pairs (i, i+3B)
    loss = sum(-pos/T) + sum(log(rowsum(exp(sim/T)) - diag)) / (6B)

Strategy (data-parallel over B):
  Each core owns B/8 = 256 batch rows -> 1536 of the 12288 z-rows
  (rows of both modalities for its batch slice, so pos pairs stay local).
  Per core, on device, per modality half (spa then seq):
    - load f shard, PE-transpose -> fT, projection matmuls -> vT half
    - column norms via ones-matmul; r = 16 * rsqrt(ssq) via exp/ln
    - zT_half = fp8_e4m3(vT * r)  [512, 768]  (x16 scaling keeps fp8 in
      normal range; folded back via the exp() scale and the pos term)
    - AllGather the half (so the spa gather overlaps the seq prologue,
      and the seq gather overlaps the first sim tiles)
  sim tiles: DoubleRow fp8 matmuls (K=256 per instruction) of
  zT_local.T @ zT_all with fused exp(sim/(T*256)) + row-sum on ACT.
  denom = rowsum - e^2 ; partial loss = sum(log denom) - (2/T)*sum(pos).
  Host sums the 8 partial scalars (the trivial all-reduce of the loss).
"""
import sys
from contextlib import ExitStack

sys.path.insert(0, "/opt/trn_rl_repo")

import numpy as np

import concourse.bass as bass
import concourse.mybir as mybir
import concourse.tile as tile
from concourse import bacc
from concourse import bass_utils
from concourse.masks import make_identity

F32 = mybir.dt.float32
BF16 = mybir.dt.bfloat16
FP8 = mybir.dt.float8e4
AF = mybir.ActivationFunctionType
ALU = mybir.AluOpType
DR = mybir.MatmulPerfMode.DoubleRow

N_CORES = 8
B = 2048
BL = B // N_CORES          # 256 local batch rows
D = 512
KB = D // 128              # 4 d blocks of 128
HROWS = 3 * BL             # 768 rows per modality half
LROWS = 2 * HROWS          # 1536 local z-rows (spa 768 | seq 768)
R = N_CORES * LROWS        # 12288 total rows
HALL = N_CORES * HROWS     # 6144 gathered columns per half
IB = LROWS // 128          # 12 row blocks of 128 per core
CC = HALL // 1536          # 4 sim column chunks of 1536 per half
LH = (0, 1, 3)             # left heads of the pairs (x, 2)
TEMP = 0.5
ZSCALE = 16.0              # fp8 z scaling
ESCALE = (1.0 / TEMP) / (ZSCALE * ZSCALE)
POS_COEF = (-2.0 / TEMP) / (ZSCALE * ZSCALE)
E2 = float(np.exp(2.0))    # diagonal term exp(2 * ||z||^2), ||z|| == 1
INV_COUNT = 1.0 / R        # final 1/(2*half)


def _body(ctx, nc, tc, f_aps, w_ap, b_ap, out_ap):
    const_pool = ctx.enter_context(tc.tile_pool(name="const", bufs=1))
    small_pool = ctx.enter_context(tc.tile_pool(name="small", bufs=1))
    vt_pool = ctx.enter_context(tc.tile_pool(name="vt", bufs=1))
    dram_pool = ctx.enter_context(tc.tile_pool(name="dram", bufs=1,
                                               space="DRAM"))
    big_pool = ctx.enter_context(tc.tile_pool(name="big", bufs=1))

    ident = const_pool.tile([128, 128], F32)
    make_identity(nc, ident[:])
    ones_col = const_pool.tile([128, 1], F32)
    nc.vector.memset(ones_col[:], 1.0)
    ones_row = const_pool.tile([1, 128], F32)
    nc.vector.memset(ones_row[:], 1.0)
    neg_e2 = const_pool.tile([128, 1], F32)
    nc.vector.memset(neg_e2[:], -E2)
    ln_zs = const_pool.tile([1, 1], F32)
    nc.vector.memset(ln_zs[:], float(np.log(ZSCALE)))

    # b columns: [128, 4] (per d_out block)
    b_col = const_pool.tile([128, 4], F32)
    for m in range(KB):
        nc.sync.dma_start(b_col[:, m:m + 1], b_ap[m * 128:(m + 1) * 128])

    w_bf = const_pool.tile([128, 8, D], BF16)

    vT = vt_pool.tile([128, KB, LROWS], F32)       # [d_out(blk,128), rows]
    zT_loc = small_pool.tile([128, KB, LROWS], FP8, tag="zT_loc")
    r_row = small_pool.tile([1, LROWS], F32, tag="r_row")
    zT_all = [None, None]

    with tc.tile_pool(name="fstage", bufs=2) as fst_pool, \
         tc.tile_pool(name="ftrans", bufs=1) as ft_pool, \
         tc.tile_pool(name="sq", bufs=2) as sq_pool, \
         tc.tile_pool(name="ps_t", bufs=2, space="PSUM") as ps_t, \
         tc.tile_pool(name="ps_proj", bufs=2, space="PSUM") as ps_proj, \
         tc.tile_pool(name="ps_s", bufs=2, space="PSUM") as ps_s:

        # W: [1024, 512] f32 -> bf16 [128, 8(kblk), 512(d_out)]
        w_st = fst_pool.tile([128, 8, D], F32, tag="w_st")
        for kb in range(8):
            nc.sync.dma_start(w_st[:, kb, :], w_ap[kb * 128:(kb + 1) * 128, :])
        nc.vector.tensor_copy(w_bf[:], w_st[:])

        for mod in range(2):                   # 0 = spa, 1 = seq
            c0 = mod * HROWS
            # ---- load + transpose f ----
            fT = ft_pool.tile([128, 4, KB, 2 * 128], BF16, name=f"fT{mod}",
                              tag=f"fT{mod}")
            for h in range(2):                 # halves of 256 local rows
                f_st = fst_pool.tile([128, 4 * D], F32, name="f_st",
                                     tag="f_st")
                nc.sync.dma_start(
                    f_st[:], f_aps[mod][h * 128:(h + 1) * 128, :, :])
                for a in range(4):
                    for kb in range(KB):
                        pst = ps_t.tile([128, 128], F32, name="pst", tag="pst")
                        nc.tensor.transpose(
                            pst[:],
                            f_st[:, a * D + kb * 128: a * D + (kb + 1) * 128],
                            ident[:])
                        nc.any.tensor_copy(
                            fT[:, a, kb, h * 128:(h + 1) * 128], pst[:])
            # ---- projection ----
            for pa in range(3):
                for m in range(KB):
                    psv = ps_proj.tile([128, 2 * 128], F32, name="psv",
                                       tag="psv")
                    for kk in range(8):
                        head = LH[pa] if kk < 4 else 2
                        kb = kk % 4
                        nc.tensor.matmul(
                            psv[:],
                            lhsT=w_bf[:, kk, m * 128:(m + 1) * 128],
                            rhs=fT[:, head, kb, :],
                            start=(kk == 0), stop=(kk == 7))
                    col0 = c0 + pa * 256
                    nc.vector.tensor_scalar_add(
                        vT[:, m, col0:col0 + 256], psv[:], b_col[:, m:m + 1])

            # ---- norms: ssq over d for this half's 768 columns ----
            ssq = small_pool.tile([1, HROWS], F32, name=f"ssq{mod}",
                                  tag=f"ssq{mod}")
            for co, cw in ((0, 512), (512, 256)):
                ps_ssq = ps_s.tile([1, 512], F32, name="ps_ssq", tag="ps_s")
                for m in range(KB):
                    sq = sq_pool.tile([128, 512], F32, name="sq", tag="sq")
                    nc.vector.tensor_mul(sq[:, :cw],
                                         vT[:, m, c0 + co:c0 + co + cw],
                                         vT[:, m, c0 + co:c0 + co + cw])
                    nc.tensor.matmul(ps_ssq[:, :cw], lhsT=ones_col[:],
                                     rhs=sq[:, :cw],
                                     start=(m == 0), stop=(m == KB - 1))
                nc.vector.tensor_copy(ssq[:, co:co + cw], ps_ssq[:, :cw])

            # r = ZSCALE / sqrt(ssq) = exp(-0.5*ln(ssq) + ln(ZSCALE))
            lnss = small_pool.tile([1, HROWS], F32, name=f"lnss{mod}",
                                   tag=f"lnss{mod}")
            nc.scalar.activation(lnss[:], ssq[:], AF.Ln)
            nc.scalar.activation(r_row[:, c0:c0 + HROWS], lnss[:], AF.Exp,
                                 scale=-0.5, bias=ln_zs[:])

            # zT_loc half = fp8(vT * r)
            for co, cw in ((0, 512), (512, 256)):
                rb = ps_s.tile([128, 512], F32, name="rb", tag="rb")
                nc.tensor.matmul(rb[:, :cw], lhsT=ones_row[:],
                                 rhs=r_row[:, c0 + co:c0 + co + cw],
                                 start=True, stop=True)
                for m in range(KB):
                    nc.vector.tensor_mul(
                        zT_loc[:, m, c0 + co:c0 + co + cw],
                        vT[:, m, c0 + co:c0 + co + cw], rb[:, :cw])

            # ---- AllGather this half ----
            ag_in = dram_pool.tile([4 * 128, HROWS], FP8, name=f"ag_in{mod}",
                                   tag=f"ag_in{mod}")
            ag_out = dram_pool.tile([N_CORES * 4 * 128, HROWS], FP8,
                                    addr_space="Shared", name=f"ag_out{mod}",
                                    tag=f"ag_out{mod}")
            for m in range(KB):
                nc.sync.dma_start(ag_in[m * 128:(m + 1) * 128, :],
                                  zT_loc[:, m, c0:c0 + HROWS])
            nc.gpsimd.collective_compute(
                "AllGather", ALU.bypass,
                replica_groups=[list(range(N_CORES))],
                ins=[ag_in.opt()], outs=[ag_out.opt()])
            zT_all[mod] = big_pool.tile([128, KB, HALL], FP8,
                                        name=f"zT_all{mod}", tag=f"zTa{mod}")
            for rr in range(N_CORES):
                for m in range(KB):
                    nc.sync.dma_start(
                        zT_all[mod][:, m, rr * HROWS:(rr + 1) * HROWS],
                        ag_out[rr * 512 + m * 128: rr * 512 + (m + 1) * 128, :])

        # ---- pos_i = r_i * r_{i+768} * sum_d vT[d, i] * vT[d, i+768] ----
        pos_raw = small_pool.tile([1, HROWS], F32, tag="pos_raw")
        for co, cw in ((0, 512), (512, 256)):
            ps_pp = ps_s.tile([1, 512], F32, name="ps_pp", tag="ps_s")
            for m in range(KB):
                pp = sq_pool.tile([128, 512], F32, name="pp", tag="sq")
                nc.vector.tensor_mul(pp[:, :cw], vT[:, m, co:co + cw],
                                     vT[:, m, HROWS + co:HROWS + co + cw])
                nc.tensor.matmul(ps_pp[:, :cw], lhsT=ones_col[:],
                                 rhs=pp[:, :cw],
                                 start=(m == 0), stop=(m == KB - 1))
            nc.vector.tensor_copy(pos_raw[:, co:co + cw], ps_pp[:, :cw])
        rrp = small_pool.tile([1, HROWS], F32, tag="rrp")
        nc.vector.tensor_mul(rrp[:], r_row[:, 0:HROWS], r_row[:, HROWS:LROWS])
        pos_row = small_pool.tile([1, HROWS], F32, tag="pos_row")
        nc.vector.tensor_mul(pos_row[:], pos_raw[:], rrp[:])
        pos_sum = small_pool.tile([1, 1], F32, tag="pos_sum")
        nc.vector.tensor_reduce(pos_sum[:], pos_row[:],
                                axis=mybir.AxisListType.X, op=ALU.add)

    # ---------- sim tiles + fused exp/rowsum (DoubleRow fp8) ----------
    stats = small_pool.tile([128, 2 * IB * CC], F32, tag="stats")
    with tc.tile_pool(name="ps_sim", bufs=2, space="PSUM") as ps_sim:
        for mod in range(2):
            for ib in range(IB):
                for cc in range(CC):
                    ps = ps_sim.tile([128, 1536], F32, name="ps_sim",
                                     tag="ps_sim")
                    for jt in range(3):
                        j0 = cc * 1536 + jt * 512
                        for g in range(2):
                            nc.tensor.matmul(
                                ps[:, jt * 512:(jt + 1) * 512],
                                lhsT=zT_loc[:, 2 * g:2 * g + 2,
                                            ib * 128:(ib + 1) * 128],
                                rhs=zT_all[mod][:, 2 * g:2 * g + 2,
                                                j0:j0 + 512],
                                start=(g == 0), stop=(g == 1),
                                perf_mode=DR)
                    nc.scalar.activation(
                        ps[:], ps[:], AF.Exp, scale=ESCALE,
                        accum_out=stats[:, (mod * IB + ib) * CC + cc:
                                        (mod * IB + ib) * CC + cc + 1])

    # ---------- final reduction ----------
    with tc.tile_pool(name="ps_fin", bufs=1, space="PSUM") as ps_fin:
        denom = small_pool.tile([128, IB], F32, tag="denom")
        nc.vector.tensor_reduce(
            denom[:], stats.rearrange("p (i r) -> p i r", r=2 * CC),
            axis=mybir.AxisListType.X, op=ALU.add)
        logd = small_pool.tile([128, IB], F32, tag="logd")
        nc.scalar.activation(logd[:], denom[:], AF.Ln, bias=neg_e2[:])
        logsum = small_pool.tile([128, 1], F32, tag="logsum")
        nc.vector.tensor_reduce(logsum[:], logd[:],
                                axis=mybir.AxisListType.X, op=ALU.add)
        fin = ps_fin.tile([1, 1], F32, tag="fin")
        nc.tensor.matmul(fin[:], lhsT=ones_col[:], rhs=logsum[:],
                         start=True, stop=True)
        res = small_pool.tile([1, 1], F32, tag="res")
        # res = (pos_sum * POS_COEF + sum(log denom)) / R
        nc.vector.scalar_tensor_tensor(res[:], pos_sum[:], POS_COEF,
                                       fin[:], op0=ALU.mult, op1=ALU.add)
        nc.vector.tensor_scalar_mul(res[:], res[:], INV_COUNT)
        nc.sync.dma_start(out_ap[:], res[:])


_NC_CACHE = None


def build_nc():
    global _NC_CACHE
    if _NC_CACHE is not None:
        return _NC_CACHE
    nc = bacc.Bacc("TRN2", target_bir_lowering=False, debug=False,
                   num_devices=N_CORES)
    f_spa = nc.dram_tensor("f_spa", [BL, 4, D], F32, kind="ExternalInput").ap()
    f_seq = nc.dram_tensor("f_seq", [BL, 4, D], F32, kind="ExternalInput").ap()
    w_ap = nc.dram_tensor("W", [2 * D, D], F32, kind="ExternalInput").ap()
    b_ap = nc.dram_tensor("b", [D], F32, kind="ExternalInput").ap()
    out_ap = nc.dram_tensor("out", [1, 1], F32, kind="ExternalOutput").ap()
    with tile.TileContext(nc) as tc, ExitStack() as ctx:
        _body(ctx, nc, tc, (f_spa, f_seq), w_ap, b_ap, out_ap)
    nc.compile()
    _NC_CACHE = nc
    return nc


def run(inputs, **kw):
    nc = build_nc()
    f_seq = np.ascontiguousarray(np.asarray(inputs["f_seq"], dtype=np.float32))
    f_spa = np.ascontiguousarray(np.asarray(inputs["f_spa"], dtype=np.float32))
    W = np.ascontiguousarray(np.asarray(inputs["W"], dtype=np.float32))
    b = np.ascontiguousarray(np.asarray(inputs["b"], dtype=np.float32))
    in_maps = []
    for c in range(N_CORES):
        sl = slice(c * BL, (c + 1) * BL)
        in_maps.append({"f_seq": np.ascontiguousarray(f_seq[sl]),
                        "f_spa": np.ascontiguousarray(f_spa[sl]),
                        "W": W, "b": b})
    res = bass_utils.run_bass_kernel_spmd(
        nc, in_maps, core_ids=list(range(N_CORES)), **kw)
    total = np.float64(0.0)
    for c in range(N_CORES):
        total += np.float64(res.results[c]["out"][0, 0])
    return np.float32(total), res


def kernel(**inputs) -> np.ndarray:
    loss, _ = run(inputs)
    return np.asarray(loss, dtype=np.float32)


if __name__ == "__main__":
    rng = np.random.default_rng(0)
    inputs = {
        "f_seq": rng.standard_normal((B, 4, D), dtype=np.float32),
        "f_spa": rng.standard_normal((B, 4, D), dtype=np.float32),
        "W": (rng.standard_normal((2 * D, D), dtype=np.float32) * 0.02),
        "b": np.zeros((D,), dtype=np.float32),
    }
    print(kernel(**inputs))
